# revision 18
# baseline (speedup 1.0000x reference)
"""Multi-head attention (B=4, S=2048, D=1024, H=16, causal) on 8 TRN2 NeuronCores.

Sharding: batch x head-group (Megatron).  Core c handles batch c//2 and head
group c%2 (8 heads = 512 of the 1024 hidden dims).  w_q/w_k/w_v are
column-parallel, w_o row-parallel; the two partial outputs per batch are summed
on the host during unsharding.

Device kernel (per core, all matmuls bf16, fp32 accumulation):
  - DMA-transpose loads of x (token-major -> dim-major)
  - qT/kT projections in [dim, token] layout; V projection in [token, dim]
    with a ones block packed next to each head's V columns
  - scoresT[k,q] = kT.T @ qT per head, two heads row-packed on the PE array
  - exp on ScalarE (scores are O(1): no max subtraction needed; causal masking
    by construction: only valid k-tiles/columns computed, triangle zeroed via
    gpsimd affine_select on the exp output)
  - attn@V with the [V | ones] stationary tile, so the softmax denominator is
    accumulated in PSUM partitions 64:128 of the same matmul for free
  - normalize with DVE reciprocal_approx_fast + mul straight into the o-proj
    operand layout
  - o-proj per query chunk, interleaved into the attention phase; b_q added on
    qT evacuation, b_k dropped (cancels in softmax), b_v folded into b_o on host
"""

import os
import sys

for _p in ("/opt/trn_rl_repo",):
    if _p not in sys.path and os.path.isdir(_p):
        sys.path.insert(0, _p)

from contextlib import ExitStack

import ml_dtypes
import numpy as np

import concourse.bass as bass
import concourse.tile as tile
from concourse import bacc, mybir
from concourse import bass_utils

BF16 = ml_dtypes.bfloat16

B = 4
S = 2048
D = 1024
H = 16
DK = 64
NCORES = 8
DL = D // 2  # local (per head-group) hidden dims = 512
NHP = 4  # head pairs per core
KT = D // 128  # contraction tiles over model dim = 8
TT = S // 128  # token tiles = 16
QC = S // 512  # query chunks of 512 = 4

FP32 = mybir.dt.float32
DTBF = mybir.dt.bfloat16


def _emit(nc, causal: bool):
    xq = nc.dram_tensor("xq_t", [D, S], DTBF, kind="ExternalInput").ap()
    xk = nc.dram_tensor("xk_t", [D, S], DTBF, kind="ExternalInput").ap()
    xv = nc.dram_tensor("xv_t", [D, S], DTBF, kind="ExternalInput").ap()
    wq_t = nc.dram_tensor("wq_p", [128, KT, DL], DTBF, kind="ExternalInput").ap()
    wk_t = nc.dram_tensor("wk_p", [128, KT, DL], DTBF, kind="ExternalInput").ap()
    wv_t = nc.dram_tensor("wv_p", [128, KT, DL], DTBF, kind="ExternalInput").ap()
    wo_t = nc.dram_tensor("wo_p", [128, NHP, D], DTBF, kind="ExternalInput").ap()
    bq_t = nc.dram_tensor("bq_t", [128, 4], FP32, kind="ExternalInput").ap()
    bo_t = nc.dram_tensor("bo_t", [128, 8], FP32, kind="ExternalInput").ap()
    out_pt = nc.dram_tensor("out_pt", [D, S], FP32, kind="ExternalOutput").ap()

    EXP = mybir.ActivationFunctionType.Exp
    LN = mybir.ActivationFunctionType.Ln

    with tile.TileContext(nc) as tc, ExitStack() as ctx:
        consts = ctx.enter_context(tc.tile_pool(name="consts", bufs=1))
        xt_pool = ctx.enter_context(tc.tile_pool(name="xt", bufs=2))
        qkv_pool = ctx.enter_context(tc.tile_pool(name="qkv", bufs=1))
        et_pool = ctx.enter_context(tc.tile_pool(name="et", bufs=4))
        rc_pool = ctx.enter_context(tc.tile_pool(name="rc", bufs=4))
        out_pool = ctx.enter_context(tc.tile_pool(name="osb", bufs=3))
        ps_s = ctx.enter_context(tc.tile_pool(name="ps_s", bufs=2, space="PSUM"))
        ps_acc = ctx.enter_context(tc.tile_pool(name="ps_acc", bufs=2, space="PSUM"))
        ps_op = ctx.enter_context(tc.tile_pool(name="ps_op", bufs=2, space="PSUM"))

        # ---- constants (scalar-engine HWDGE ring; transposes use sync) -----
        wq_sb = consts.tile([128, KT, DL], DTBF)
        nc.gpsimd.dma_start(wq_sb[:], wq_t[:])
        wk_sb = consts.tile([128, KT, DL], DTBF)
        nc.gpsimd.dma_start(wk_sb[:], wk_t[:])
        wv_sb = consts.tile([128, KT, DL], DTBF)
        nc.gpsimd.dma_start(wv_sb[:], wv_t[:])
        wo_sb = consts.tile([128, NHP, D], DTBF)
        nc.gpsimd.dma_start(wo_sb[:], wo_t[:])
        bq_sb = consts.tile([128, 4], FP32)
        nc.gpsimd.dma_start(bq_sb[:], bq_t[:])
        bo_sb = consts.tile([128, 8], FP32)
        nc.gpsimd.dma_start(bo_sb[:], bo_t[:])

        qT_sb = qkv_pool.tile([128, NHP, S], DTBF)
        kT_sb = qkv_pool.tile([128, NHP, S], DTBF)
        # [V | ones] per head: vp_sb[:, tt, h, 0:64] = V, [.., 64:128] = 1.0
        vp_sb = qkv_pool.tile([128, TT, 8, 128], DTBF)
        a_sb = qkv_pool.tile([128, NHP, S], DTBF)

        for h in range(8):
            nc.vector.memset(vp_sb[:, :, h, 64:128], 1.0)

        # lower-triangle-inclusive (k <= q) binary mask for diagonal tiles
        tri_sb = consts.tile([128, 128], DTBF)
        nc.gpsimd.memset(tri_sb[:], 1.0)
        nc.gpsimd.affine_select(
            out=tri_sb[:],
            in_=tri_sb[:],
            compare_op=mybir.AluOpType.is_ge,
            fill=0.0,
            base=0,
            pattern=[[1, 128]],
            channel_multiplier=-1,
        )

        # ---- projections ---------------------------------------------------
        def load_xt(x_dram):
            # x_dram is [D, S] (host pre-transposed); row block k*128.. maps
            # straight onto partitions
            xt = xt_pool.tile([128, KT, S], DTBF, tag="xt", name="xt")
            for k in range(KT):
                eng = nc.sync if k % 2 == 0 else nc.scalar
                eng.dma_start(xt[:, k, :], x_dram[k * 128:(k + 1) * 128, :])
            return xt

        def proj_dim_major(xt, w_sb, dst, bias):
            # dst[:, ot, t] (dim-major): lhsT = w tile, rhs = xT
            for ot in range(4):
                for tc4 in range(4):
                    ps = ps_op.tile([128, 512], FP32, tag="op", name="ps")
                    for k in range(KT):
                        nc.tensor.matmul(
                            ps[:],
                            w_sb[:, k, ot * 128:(ot + 1) * 128],
                            xt[:, k, tc4 * 512:(tc4 + 1) * 512],
                            start=(k == 0),
                            stop=(k == KT - 1),
                        )
                    if bias is not None:
                        nc.vector.tensor_scalar_add(
                            dst[:, ot, tc4 * 512:(tc4 + 1) * 512], ps[:], bias[:, ot:ot + 1]
                        )
                    else:
                        nc.vector.tensor_copy(dst[:, ot, tc4 * 512:(tc4 + 1) * 512], ps[:])

        xt = load_xt(xq)
        proj_dim_major(xt, wq_sb, qT_sb, bq_sb)
        xtk = load_xt(xk)

        def kproj(ot):
            for tc4 in range(4):
                ps = ps_op.tile([128, 512], FP32, tag="op", name="ps")
                for k in range(KT):
                    nc.tensor.matmul(
                        ps[:],
                        wk_sb[:, k, ot * 128:(ot + 1) * 128],
                        xtk[:, k, tc4 * 512:(tc4 + 1) * 512],
                        start=(k == 0),
                        stop=(k == KT - 1),
                    )
                nc.vector.tensor_copy(kT_sb[:, ot, tc4 * 512:(tc4 + 1) * 512], ps[:])

        kproj(0)
        xt = load_xt(xv)

        # V in token-major layout: lhsT = xT tile (stationary), rhs = w
        def vproj(tt):
            ps = ps_op.tile([128, 512], FP32, tag="op", name="ps")
            for k in range(KT):
                nc.tensor.matmul(
                    ps[:],
                    xt[:, k, tt * 128:(tt + 1) * 128],
                    wv_sb[:, k, :],
                    start=(k == 0),
                    stop=(k == KT - 1),
                )
            nc.vector.tensor_copy(vp_sb[:, tt, :, 0:64], ps[:])

        # ---- attention + interleaved output projection ---------------------
        # v-proj for token quartet qc is emitted just before the attention
        # chunk that first needs it, so PE fills ACT-bound gaps
        fill_q = []

        def attn(qc, hp):
            if True:
                jmax = 4 * qc + 3 if causal else TT - 1
                q0 = qc * 512
                pso = [ps_acc.tile([128, 512], FP32, tag="acc", name=f"pso{h2}") for h2 in range(2)]

                def offof(j):
                    r = j - 4 * qc if causal else -1
                    return 128 * r if r >= 0 else 0

                def scores(j):
                    off = offof(j)
                    pss = ps_s.tile([128, 2, 512], FP32, tag="ps_s", name="pss")
                    for h2 in range(2):
                        nc.tensor.matmul(
                            pss[:, h2, off:512],
                            kT_sb[h2 * 64:(h2 + 1) * 64, hp, j * 128:(j + 1) * 128],
                            qT_sb[h2 * 64:(h2 + 1) * 64, hp, q0 + off:q0 + 512],
                            start=True,
                            stop=True,
                        )
                    et = et_pool.tile([128, 2, 512], DTBF, tag="et", name="et")
                    nc.scalar.activation(et[:, :, off:], pss[:, :, off:], EXP, scale=0.125)
                    if off or (causal and j == 4 * qc):
                        # zero where k (partition) > q (free col)
                        for h2 in range(2):
                            nc.vector.tensor_mul(
                                et[:, h2, off:off + 128],
                                et[:, h2, off:off + 128],
                                tri_sb[:],
                            )
                    return et

                et_next = scores(0)
                for j in range(jmax + 1):
                    off = offof(j)
                    et = et_next
                    if j < jmax:
                        et_next = scores(j + 1)
                    for h2 in range(2):
                        # rows 0:64 accumulate attn@V, rows 64:128 the softmax
                        # denominator (ones block).  Causally-trimmed widths on
                        # interleaved chains; per-element has_written semantics
                        # make this safe but the sim's zero-region tracker
                        # can't express it.
                        nc.tensor.matmul(
                            pso[h2][:, off:512],
                            vp_sb[:, j, 2 * hp + h2, :],
                            et[:, h2, off:],
                            start=(j == 0),
                            stop=(j == jmax),
                            skip_group_check=True,
                        )
                    if fill_q:
                        fill_q.pop(0)()
                for h2 in range(2):
                    rl = rc_pool.tile([128, 512], FP32, tag="rc", name="rl")
                    nc.scalar.activation(rl[64:128, :], pso[h2][64:128, :], LN)
                    rc = rc_pool.tile([128, 512], FP32, tag="rc", name="rc")
                    nc.scalar.activation(rc[64:128, :], rl[64:128, :], EXP, scale=-1.0)
                    nc.vector.tensor_mul(
                        a_sb[h2 * 64:(h2 + 1) * 64, hp, qc * 512:(qc + 1) * 512],
                        pso[h2][0:64, :],
                        rc[64:128, :],
                    )
        def oproj_od(qc, od):
            ps = ps_op.tile([128, 512], FP32, tag="op", name="ps")
            for hp in range(NHP):
                nc.tensor.matmul(
                    ps[:],
                    wo_sb[:, hp, od * 128:(od + 1) * 128],
                    a_sb[:, hp, qc * 512:(qc + 1) * 512],
                    start=(hp == 0),
                    stop=(hp == NHP - 1),
                )
            osb = out_pool.tile([128, 512], FP32, tag="osb", name="osb")
            nc.vector.tensor_scalar_add(osb[:], ps[:], bo_sb[:, od:od + 1])
            nc.sync.dma_start(
                out_pt[od * 128:(od + 1) * 128, qc * 512:(qc + 1) * 512], osb[:]
            )

        for qc in range(QC):
            if qc == 0:
                for tt in range(4):
                    vproj(tt)
                for hp in range(NHP):
                    attn(qc, hp)
                    if hp < NHP - 1:
                        kproj(hp + 1)
            else:
                fill_q.extend(
                    (lambda tt=tt: vproj(tt)) for tt in range(4 * qc, 4 * qc + 4)
                )
                fill_q.extend(
                    (lambda od=od: oproj_od(qc - 1, od)) for od in range(8)
                )
                for hp in range(NHP):
                    attn(qc, hp)
                while fill_q:
                    fill_q.pop(0)()
        for od in range(8):
            oproj_od(QC - 1, od)


_CACHE = {}


def _patched_act_tables(arch):
    """Make the combined Ln+Exp set the only one advertising Exp/Ln so the
    table-load pass picks it everywhere (one load, no set thrashing).  Set
    positions (= act_func_set_id) are preserved."""
    t = dict(_orig_act_tables(arch))
    name = "natural_log_exp_and_others"
    if name in t:
        exp_ln = {f for f in t[name] if f.name in ("Exp", "Ln")}
        t = {
            k: (v if k == name else (set(v) - exp_ln))
            for k, v in t.items()
        }
    return t


_orig_act_tables = bacc.get_activation_tables
bacc.get_activation_tables = _patched_act_tables


def _get_compiled(causal: bool):
    key = bool(causal)
    if key not in _CACHE:
        nc = bacc.Bacc("TRN2", target_bir_lowering=False, debug=False, num_devices=NCORES)
        _emit(nc, causal=key)
        nc.compile()
        _CACHE[key] = nc
    return _CACHE[key]


def make_in_maps(query, key, value, w_q, b_q, w_k, b_k, w_v, b_v, w_o, b_o):
    """Build the per-core input maps (host-side sharding + layout prep)."""
    in_maps = []
    # b_v folds into the output bias: softmax rows sum to 1, so
    # attn(V + b_v) = attn(V) + b_v, and (A + b_v) @ w_o.T = A @ w_o.T + w_o @ b_v.
    # b_k drops entirely: scores shift constant along k cancels in softmax.
    bo_eff = (b_o + w_o.astype(np.float64) @ b_v.astype(np.float64)).astype(np.float32)
    for c in range(NCORES):
        b, hg = divmod(c, 2)
        sl = slice(hg * DL, (hg + 1) * DL)
        bo_core = bo_eff if hg == 0 else np.zeros_like(bo_eff)
        in_maps.append(
            {
                "xq_t": np.ascontiguousarray(query[b].T).astype(BF16),
                "xk_t": np.ascontiguousarray(key[b].T).astype(BF16),
                "xv_t": np.ascontiguousarray(value[b].T).astype(BF16),
                "wq_p": np.ascontiguousarray(
                    w_q[sl, :].T.reshape(KT, 128, DL).transpose(1, 0, 2)).astype(BF16),
                "wk_p": np.ascontiguousarray(
                    w_k[sl, :].T.reshape(KT, 128, DL).transpose(1, 0, 2)).astype(BF16),
                "wv_p": np.ascontiguousarray(
                    w_v[sl, :].T.reshape(KT, 128, DL).transpose(1, 0, 2)).astype(BF16),
                "wo_p": np.ascontiguousarray(
                    w_o[:, sl].T.reshape(NHP, 128, D).transpose(1, 0, 2)).astype(BF16),
                "bq_t": np.ascontiguousarray(b_q[sl].reshape(4, 128).T).astype(np.float32),
                "bo_t": np.ascontiguousarray(bo_core.reshape(8, 128).T).astype(np.float32),
            }
        )
    return in_maps


def _mask_is_causal(mask):
    m = np.asarray(mask).reshape(S, S)
    return bool(np.array_equal(m, np.triu(np.ones((S, S), bool), k=1)))


def _mask_is_empty(mask):
    return not np.asarray(mask).any()


def kernel(query, key, value, mask, w_q, b_q, w_k, b_k, w_v, b_v, w_o, b_o, **_unused):
    query = np.asarray(query, np.float32)
    key = np.asarray(key, np.float32)
    value = np.asarray(value, np.float32)
    if _mask_is_causal(mask):
        causal = True
    elif _mask_is_empty(mask):
        causal = False
    else:
        raise NotImplementedError("only causal or empty masks are supported")

    nc = _get_compiled(causal)
    in_maps = make_in_maps(
        query, key, value,
        np.asarray(w_q, np.float32), np.asarray(b_q, np.float32),
        np.asarray(w_k, np.float32), np.asarray(b_k, np.float32),
        np.asarray(w_v, np.float32), np.asarray(b_v, np.float32),
        np.asarray(w_o, np.float32), np.asarray(b_o, np.float32),
    )
    res = bass_utils.run_bass_kernel_spmd(nc, in_maps, core_ids=list(range(NCORES)))
    out = np.empty((B, S, D), np.float32)
    for b in range(B):
        acc = res.results[2 * b]["out_pt"] + res.results[2 * b + 1]["out_pt"]
        out[b] = acc.T
    return out


# revision 19
# speedup vs baseline: 1.0120x; 1.0120x over previous
"""Multi-head attention (B=4, S=2048, D=1024, H=16, causal) on 8 TRN2 NeuronCores.

Sharding: batch x head-group (Megatron).  Core c handles batch c//2 and head
group c%2 (8 heads = 512 of the 1024 hidden dims).  w_q/w_k/w_v are
column-parallel, w_o row-parallel; the two partial outputs per batch are summed
on the host during unsharding.

Device kernel (per core, all matmuls bf16, fp32 accumulation):
  - DMA-transpose loads of x (token-major -> dim-major)
  - qT/kT projections in [dim, token] layout; V projection in [token, dim]
    with a ones block packed next to each head's V columns
  - scoresT[k,q] = kT.T @ qT per head, two heads row-packed on the PE array
  - exp on ScalarE (scores are O(1): no max subtraction needed; causal masking
    by construction: only valid k-tiles/columns computed, triangle zeroed via
    gpsimd affine_select on the exp output)
  - attn@V with the [V | ones] stationary tile, so the softmax denominator is
    accumulated in PSUM partitions 64:128 of the same matmul for free
  - normalize with DVE reciprocal_approx_fast + mul straight into the o-proj
    operand layout
  - o-proj per query chunk, interleaved into the attention phase; b_q added on
    qT evacuation, b_k dropped (cancels in softmax), b_v folded into b_o on host
"""

import os
import sys

for _p in ("/opt/trn_rl_repo",):
    if _p not in sys.path and os.path.isdir(_p):
        sys.path.insert(0, _p)

from contextlib import ExitStack

import ml_dtypes
import numpy as np

import concourse.bass as bass
import concourse.tile as tile
from concourse import bacc, mybir
from concourse import bass_utils

BF16 = ml_dtypes.bfloat16

B = 4
S = 2048
D = 1024
H = 16
DK = 64
NCORES = 8
DL = D // 2  # local (per head-group) hidden dims = 512
NHP = 4  # head pairs per core
KT = D // 128  # contraction tiles over model dim = 8
TT = S // 128  # token tiles = 16
QC = S // 512  # query chunks of 512 = 4

FP32 = mybir.dt.float32
DTBF = mybir.dt.bfloat16


def _emit(nc, causal: bool):
    xq = nc.dram_tensor("xq_t", [D, S], DTBF, kind="ExternalInput").ap()
    xk = nc.dram_tensor("xk_t", [D, S], DTBF, kind="ExternalInput").ap()
    xv = nc.dram_tensor("xv_t", [D, S], DTBF, kind="ExternalInput").ap()
    wq_t = nc.dram_tensor("wq_p", [128, KT, DL], DTBF, kind="ExternalInput").ap()
    wk_t = nc.dram_tensor("wk_p", [128, KT, DL], DTBF, kind="ExternalInput").ap()
    wv_t = nc.dram_tensor("wv_p", [128, KT, DL], DTBF, kind="ExternalInput").ap()
    wo_t = nc.dram_tensor("wo_p", [128, NHP, D], DTBF, kind="ExternalInput").ap()
    bq_t = nc.dram_tensor("bq_t", [128, 4], FP32, kind="ExternalInput").ap()
    bo_t = nc.dram_tensor("bo_t", [128, 8], FP32, kind="ExternalInput").ap()
    out_pt = nc.dram_tensor("out_pt", [D, S], FP32, kind="ExternalOutput").ap()

    EXP = mybir.ActivationFunctionType.Exp
    LN = mybir.ActivationFunctionType.Ln

    with tile.TileContext(nc) as tc, ExitStack() as ctx:
        consts = ctx.enter_context(tc.tile_pool(name="consts", bufs=1))
        xt_pool = ctx.enter_context(tc.tile_pool(name="xt", bufs=2))
        qkv_pool = ctx.enter_context(tc.tile_pool(name="qkv", bufs=1))
        et_pool = ctx.enter_context(tc.tile_pool(name="et", bufs=4))
        rc_pool = ctx.enter_context(tc.tile_pool(name="rc", bufs=4))
        out_pool = ctx.enter_context(tc.tile_pool(name="osb", bufs=3))
        ps_s = ctx.enter_context(tc.tile_pool(name="ps_s", bufs=2, space="PSUM"))
        ps_acc = ctx.enter_context(tc.tile_pool(name="ps_acc", bufs=2, space="PSUM"))
        ps_op = ctx.enter_context(tc.tile_pool(name="ps_op", bufs=2, space="PSUM"))

        # ---- constants (scalar-engine HWDGE ring; transposes use sync) -----
        wq_sb = consts.tile([128, KT, DL], DTBF)
        for k in range(KT):
            nc.sync.dma_start(wq_sb[:, k, :], wq_t[:, k, :])
        wk_sb = consts.tile([128, KT, DL], DTBF)
        nc.gpsimd.dma_start(wk_sb[:], wk_t[:])
        wv_sb = consts.tile([128, KT, DL], DTBF)
        nc.gpsimd.dma_start(wv_sb[:], wv_t[:])
        wo_sb = consts.tile([128, NHP, D], DTBF)
        nc.gpsimd.dma_start(wo_sb[:], wo_t[:])
        bq_sb = consts.tile([128, 4], FP32)
        nc.gpsimd.dma_start(bq_sb[:], bq_t[:])
        bo_sb = consts.tile([128, 8], FP32)
        nc.gpsimd.dma_start(bo_sb[:], bo_t[:])

        qT_sb = qkv_pool.tile([128, NHP, S], DTBF)
        kT_sb = qkv_pool.tile([128, NHP, S], DTBF)
        # [V | ones] per head: vp_sb[:, tt, h, 0:64] = V, [.., 64:128] = 1.0
        vp_sb = qkv_pool.tile([128, TT, 8, 128], DTBF)
        a_sb = qkv_pool.tile([128, NHP, S], DTBF)

        for h in range(8):
            nc.vector.memset(vp_sb[:, :, h, 64:128], 1.0)

        # lower-triangle-inclusive (k <= q) binary mask for diagonal tiles
        tri_sb = consts.tile([128, 128], DTBF)
        nc.gpsimd.memset(tri_sb[:], 1.0)
        nc.gpsimd.affine_select(
            out=tri_sb[:],
            in_=tri_sb[:],
            compare_op=mybir.AluOpType.is_ge,
            fill=0.0,
            base=0,
            pattern=[[1, 128]],
            channel_multiplier=-1,
        )

        # ---- projections ---------------------------------------------------
        def load_xt(x_dram):
            # x_dram is [D, S] (host pre-transposed); row block k*128.. maps
            # straight onto partitions
            xt = xt_pool.tile([128, KT, S], DTBF, tag="xt", name="xt")
            for k in range(KT):
                eng = nc.sync if k % 2 == 0 else nc.scalar
                eng.dma_start(xt[:, k, :], x_dram[k * 128:(k + 1) * 128, :])
            return xt

        def proj_dim_major(xt, w_sb, dst, bias):
            # dst[:, ot, t] (dim-major): lhsT = w tile, rhs = xT
            for ot in range(4):
                for tc4 in range(4):
                    ps = ps_op.tile([128, 512], FP32, tag="op", name="ps")
                    for k in range(KT):
                        nc.tensor.matmul(
                            ps[:],
                            w_sb[:, k, ot * 128:(ot + 1) * 128],
                            xt[:, k, tc4 * 512:(tc4 + 1) * 512],
                            start=(k == 0),
                            stop=(k == KT - 1),
                        )
                    if bias is not None:
                        nc.vector.tensor_scalar_add(
                            dst[:, ot, tc4 * 512:(tc4 + 1) * 512], ps[:], bias[:, ot:ot + 1]
                        )
                    else:
                        nc.vector.tensor_copy(dst[:, ot, tc4 * 512:(tc4 + 1) * 512], ps[:])

        xt = load_xt(xq)
        proj_dim_major(xt, wq_sb, qT_sb, bq_sb)
        xtk = load_xt(xk)

        def kproj(ot):
            for tc4 in range(4):
                ps = ps_op.tile([128, 512], FP32, tag="op", name="ps")
                for k in range(KT):
                    nc.tensor.matmul(
                        ps[:],
                        wk_sb[:, k, ot * 128:(ot + 1) * 128],
                        xtk[:, k, tc4 * 512:(tc4 + 1) * 512],
                        start=(k == 0),
                        stop=(k == KT - 1),
                    )
                nc.vector.tensor_copy(kT_sb[:, ot, tc4 * 512:(tc4 + 1) * 512], ps[:])

        kproj(0)
        xt = load_xt(xv)

        # V in token-major layout: lhsT = xT tile (stationary), rhs = w
        def vproj(tt):
            ps = ps_op.tile([128, 512], FP32, tag="op", name="ps")
            for k in range(KT):
                nc.tensor.matmul(
                    ps[:],
                    xt[:, k, tt * 128:(tt + 1) * 128],
                    wv_sb[:, k, :],
                    start=(k == 0),
                    stop=(k == KT - 1),
                )
            nc.vector.tensor_copy(vp_sb[:, tt, :, 0:64], ps[:])

        # ---- attention + interleaved output projection ---------------------
        # v-proj for token quartet qc is emitted just before the attention
        # chunk that first needs it, so PE fills ACT-bound gaps
        fill_q = []

        def attn(qc, hp):
            if True:
                jmax = 4 * qc + 3 if causal else TT - 1
                q0 = qc * 512
                pso = ps_acc.tile([128, 2, 512], FP32, tag="acc", name="pso", bufs=1)

                def offof(j):
                    r = j - 4 * qc if causal else -1
                    return 128 * r if r >= 0 else 0

                def scores(j):
                    off = offof(j)
                    pss = ps_s.tile([128, 2, 512], FP32, tag="ps_s", name="pss")
                    for h2 in range(2):
                        nc.tensor.matmul(
                            pss[:, h2, off:512],
                            kT_sb[h2 * 64:(h2 + 1) * 64, hp, j * 128:(j + 1) * 128],
                            qT_sb[h2 * 64:(h2 + 1) * 64, hp, q0 + off:q0 + 512],
                            start=True,
                            stop=True,
                        )
                    et = et_pool.tile([128, 2, 512], DTBF, tag="et", name="et")
                    nc.scalar.activation(et[:, :, off:], pss[:, :, off:], EXP, scale=0.125)
                    if off or (causal and j == 4 * qc):
                        # zero where k (partition) > q (free col)
                        for h2 in range(2):
                            nc.vector.tensor_mul(
                                et[:, h2, off:off + 128],
                                et[:, h2, off:off + 128],
                                tri_sb[:],
                            )
                    return et

                et_next = scores(0)
                for j in range(jmax + 1):
                    off = offof(j)
                    et = et_next
                    if j < jmax:
                        et_next = scores(j + 1)
                    for h2 in range(2):
                        # rows 0:64 accumulate attn@V, rows 64:128 the softmax
                        # denominator (ones block).  Causally-trimmed widths on
                        # interleaved chains; per-element has_written semantics
                        # make this safe but the sim's zero-region tracker
                        # can't express it.
                        nc.tensor.matmul(
                            pso[:, h2, off:512],
                            vp_sb[:, j, 2 * hp + h2, :],
                            et[:, h2, off:],
                            start=(j == 0),
                            stop=(j == jmax),
                            skip_group_check=True,
                        )
                    if fill_q:
                        fill_q.pop(0)()
                rl = rc_pool.tile([128, 2, 512], FP32, tag="rc", name="rl")
                nc.scalar.activation(rl[64:128, :, :], pso[64:128, :, :], LN)
                rc = rc_pool.tile([128, 2, 512], FP32, tag="rc", name="rc")
                nc.scalar.activation(rc[64:128, :, :], rl[64:128, :, :], EXP, scale=-1.0)
                for h2 in range(2):
                    nc.vector.tensor_mul(
                        a_sb[h2 * 64:(h2 + 1) * 64, hp, qc * 512:(qc + 1) * 512],
                        pso[0:64, h2, :],
                        rc[64:128, h2, :],
                    )
        def oproj_od(qc, od):
            ps = ps_op.tile([128, 512], FP32, tag="op", name="ps")
            for hp in range(NHP):
                nc.tensor.matmul(
                    ps[:],
                    wo_sb[:, hp, od * 128:(od + 1) * 128],
                    a_sb[:, hp, qc * 512:(qc + 1) * 512],
                    start=(hp == 0),
                    stop=(hp == NHP - 1),
                )
            osb = out_pool.tile([128, 512], FP32, tag="osb", name="osb")
            nc.vector.tensor_scalar_add(osb[:], ps[:], bo_sb[:, od:od + 1])
            nc.sync.dma_start(
                out_pt[od * 128:(od + 1) * 128, qc * 512:(qc + 1) * 512], osb[:]
            )

        for qc in range(QC):
            if qc == 0:
                for tt in range(4):
                    vproj(tt)
                for hp in range(NHP):
                    attn(qc, hp)
                    if hp < NHP - 1:
                        kproj(hp + 1)
            else:
                fill_q.extend(
                    (lambda tt=tt: vproj(tt)) for tt in range(4 * qc, 4 * qc + 4)
                )
                fill_q.extend(
                    (lambda od=od: oproj_od(qc - 1, od)) for od in range(8)
                )
                for hp in range(NHP):
                    attn(qc, hp)
                while fill_q:
                    fill_q.pop(0)()
        for od in range(8):
            oproj_od(QC - 1, od)


_CACHE = {}


def _patched_act_tables(arch):
    """Make the combined Ln+Exp set the only one advertising Exp/Ln so the
    table-load pass picks it everywhere (one load, no set thrashing).  Set
    positions (= act_func_set_id) are preserved."""
    t = dict(_orig_act_tables(arch))
    name = "natural_log_exp_and_others"
    if name in t:
        exp_ln = {f for f in t[name] if f.name in ("Exp", "Ln")}
        t = {
            k: (v if k == name else (set(v) - exp_ln))
            for k, v in t.items()
        }
    return t


_orig_act_tables = bacc.get_activation_tables
bacc.get_activation_tables = _patched_act_tables


def _get_compiled(causal: bool):
    key = bool(causal)
    if key not in _CACHE:
        nc = bacc.Bacc("TRN2", target_bir_lowering=False, debug=False, num_devices=NCORES)
        _emit(nc, causal=key)
        nc.compile()
        _CACHE[key] = nc
    return _CACHE[key]


def make_in_maps(query, key, value, w_q, b_q, w_k, b_k, w_v, b_v, w_o, b_o):
    """Build the per-core input maps (host-side sharding + layout prep)."""
    in_maps = []
    # b_v folds into the output bias: softmax rows sum to 1, so
    # attn(V + b_v) = attn(V) + b_v, and (A + b_v) @ w_o.T = A @ w_o.T + w_o @ b_v.
    # b_k drops entirely: scores shift constant along k cancels in softmax.
    bo_eff = (b_o + w_o.astype(np.float64) @ b_v.astype(np.float64)).astype(np.float32)
    for c in range(NCORES):
        b, hg = divmod(c, 2)
        sl = slice(hg * DL, (hg + 1) * DL)
        bo_core = bo_eff if hg == 0 else np.zeros_like(bo_eff)
        in_maps.append(
            {
                "xq_t": np.ascontiguousarray(query[b].T).astype(BF16),
                "xk_t": np.ascontiguousarray(key[b].T).astype(BF16),
                "xv_t": np.ascontiguousarray(value[b].T).astype(BF16),
                "wq_p": np.ascontiguousarray(
                    w_q[sl, :].T.reshape(KT, 128, DL).transpose(1, 0, 2)).astype(BF16),
                "wk_p": np.ascontiguousarray(
                    w_k[sl, :].T.reshape(KT, 128, DL).transpose(1, 0, 2)).astype(BF16),
                "wv_p": np.ascontiguousarray(
                    w_v[sl, :].T.reshape(KT, 128, DL).transpose(1, 0, 2)).astype(BF16),
                "wo_p": np.ascontiguousarray(
                    w_o[:, sl].T.reshape(NHP, 128, D).transpose(1, 0, 2)).astype(BF16),
                "bq_t": np.ascontiguousarray(b_q[sl].reshape(4, 128).T).astype(np.float32),
                "bo_t": np.ascontiguousarray(bo_core.reshape(8, 128).T).astype(np.float32),
            }
        )
    return in_maps


def _mask_is_causal(mask):
    m = np.asarray(mask).reshape(S, S)
    return bool(np.array_equal(m, np.triu(np.ones((S, S), bool), k=1)))


def _mask_is_empty(mask):
    return not np.asarray(mask).any()


def kernel(query, key, value, mask, w_q, b_q, w_k, b_k, w_v, b_v, w_o, b_o, **_unused):
    query = np.asarray(query, np.float32)
    key = np.asarray(key, np.float32)
    value = np.asarray(value, np.float32)
    if _mask_is_causal(mask):
        causal = True
    elif _mask_is_empty(mask):
        causal = False
    else:
        raise NotImplementedError("only causal or empty masks are supported")

    nc = _get_compiled(causal)
    in_maps = make_in_maps(
        query, key, value,
        np.asarray(w_q, np.float32), np.asarray(b_q, np.float32),
        np.asarray(w_k, np.float32), np.asarray(b_k, np.float32),
        np.asarray(w_v, np.float32), np.asarray(b_v, np.float32),
        np.asarray(w_o, np.float32), np.asarray(b_o, np.float32),
    )
    res = bass_utils.run_bass_kernel_spmd(nc, in_maps, core_ids=list(range(NCORES)))
    out = np.empty((B, S, D), np.float32)
    for b in range(B):
        acc = res.results[2 * b]["out_pt"] + res.results[2 * b + 1]["out_pt"]
        out[b] = acc.T
    return out
